# revision 13
# baseline (speedup 1.0000x reference)
import sys

sys.path.insert(0, "/opt/trn_rl_repo")

import numpy as np
import ml_dtypes

from concourse import bass, bacc, tile, mybir
from concourse.bass_utils import run_bass_kernel_spmd

B, S, N, D = 4, 96, 512, 8
H = 64
OUT = 24
NT = N // 128  # 4 node tiles of 128 partitions
F = H + D     # 72 features in v = [h | x]
FB = F + 1    # +1 ones row for bias

BF16 = mybir.dt.bfloat16
FP32 = mybir.dt.float32

_CACHE = {}

# All state is kept scaled by 2 (V holds 2h, C2 holds 2c) so that the whole
# LSTM tail runs off a single tanh per gate block:
#   sigmoid(x) = (tanh(x/2) + 1) / 2
# One ACT tanh(G*0.5) yields t_i,t_f,t_o,tanh(gg) at once (gg weight cols are
# pre-doubled on the host), and the (t+1)/2 factors fold into the
# scalar_tensor_tensor ops:
#   2c' = 0.5*[(t_f+1)*2c] + (t_i+1)*t_g
#   2h' = (t_o+1)*tanh(2c' * 0.5)


def _build_nc():
    nc = bacc.Bacc(None)
    adjT_d = nc.dram_tensor("adjT", [S, 128, NT, N], BF16, kind="ExternalInput")
    xT_d = nc.dram_tensor("xT", [128, S, NT, D], BF16, kind="ExternalInput")
    wb_d = nc.dram_tensor("wb", [FB, 4 * H], BF16, kind="ExternalInput")
    h0_d = nc.dram_tensor("h0T", [128, NT, H], BF16, kind="ExternalInput")
    c0_d = nc.dram_tensor("c0T", [128, NT, H], FP32, kind="ExternalInput")
    hout_d = nc.dram_tensor("hout", [128, NT, H], FP32, kind="ExternalOutput")

    TANH = mybir.ActivationFunctionType.Tanh
    COPY = mybir.ActivationFunctionType.Copy
    ADD = mybir.AluOpType.add
    MUL = mybir.AluOpType.mult

    with tile.TileContext(nc) as tc:
        with (
            tc.tile_pool(name="persist", bufs=1) as persist,
            tc.tile_pool(name="adj", bufs=3) as adjp,
            tc.tile_pool(name="scratch", bufs=2) as scratch,
            tc.tile_pool(name="ps_av", bufs=1, space="PSUM") as ps_av,
            tc.tile_pool(name="ps_g", bufs=2, space="PSUM") as ps_g,
            tc.tile_pool(name="ps_junk", bufs=1, space="PSUM") as ps_junk,
            tc.tile_pool(name="ps_c", bufs=1, space="PSUM") as ps_c,
        ):
            X = persist.tile([128, S, NT, D], BF16)   # all timesteps of x
            V = persist.tile([128, NT, F], BF16)      # [2h | x] per node tile
            C2 = ps_c.tile([128, NT, H], FP32)        # 2*cell state (PSUM:
            # ACT reads PSUM faster than SBUF-fp32, and TC->h is on the chain)
            WB = persist.tile([FB, 4 * H], BF16)      # [Wh'; Wx'; b']
            AVT = persist.tile([FB, N], BF16)         # Av^T + ones row
            HF = persist.tile([128, NT, H], FP32)     # final 2h, fp32
            H0 = persist.tile([128, NT, H], BF16)
            JS = persist.tile([128, 256], BF16)       # warmer matmul source
            JP = ps_junk.tile([128, 512], FP32)       # warmer matmul sink

            nc.gpsimd.dma_start(X[:], xT_d[:])
            nc.gpsimd.dma_start(WB[:], wb_d[:])
            nc.gpsimd.dma_start(H0[:], h0_d[:])
            C0S = persist.tile([128, NT, H], FP32)
            nc.gpsimd.dma_start(C0S[:], c0_d[:])
            nc.vector.tensor_copy(C2[:], C0S[:])  # stage c0 via SBUF into PSUM
            # all V producers stay on DVE so matmul LDW needs a single wait
            nc.vector.tensor_copy(V[:, :, 0:H], H0[:])
            nc.vector.memset(JS[:], 0.0)
            # ones row (72) for bias; partition offset must be mult of 32, so
            # memset 64:73 once — rows 64:72 are rewritten with data each step.
            nc.vector.memset(AVT[64:FB, :], 1.0)

            for s in range(S):
                AT = adjp.tile([128, NT, N], BF16, name="AT", tag="AT")
                nc.sync.dma_start(AT[:], adjT_d[s])

                # x_s into V x slots (SBUF -> SBUF), on Pool to keep DVE free
                nc.gpsimd.tensor_copy(V[:, :, H : H + D], X[:, s, :, :])

                AvT0 = ps_av.tile([FB, 256], FP32, name="AvT0", tag="AvT0")
                AvT1 = ps_av.tile([FB, 256], FP32, name="AvT1", tag="AvT1")
                GA = ps_g.tile([128, 2, 4 * H], FP32, name="GA", tag="GA")
                GB = ps_g.tile([128, 2, 4 * H], FP32, name="GB", tag="GB")

                # mm1 col-half 0; mt0/1 start as soon as h tiles 0:2 land,
                # mt2/3 wait on h tiles 2:4 (warmers below bridge the gap)
                for mt in range(2):
                    nc.tensor.matmul(
                        AvT0[0:F, :], V[:, mt, :], AT[:, mt, 0:256],
                        start=(mt == 0), stop=False,
                    )
                for mt in range(2, NT):
                    nc.tensor.matmul(
                        AvT0[0:F, :], V[:, mt, :], AT[:, mt, 0:256],
                        start=False, stop=(mt == NT - 1),
                    )
                for _ in range(2):
                    nc.tensor.matmul(
                        JP[:, 0:16], JS[:, 0:128], V[:, 3, 0:16],
                        start=True, stop=True,
                    )
                nc.vector.tensor_copy(AVT[0:F, 0:256], AvT0[0:F, :])
                for mt in range(NT):
                    nc.tensor.matmul(
                        AvT1[0:F, :], V[:, mt, :], AT[:, mt, 256:512],
                        start=(mt == 0), stop=(mt == NT - 1),
                    )
                nc.scalar.activation(AVT[0:F, 256:512], AvT1[0:F, :], COPY)

                for nt in range(2):
                    nc.tensor.matmul(
                        GA[:, nt, :], AVT[:, nt * 128 : (nt + 1) * 128], WB[:],
                        start=True, stop=True,
                    )
                for nt in range(2, NT):
                    nc.tensor.matmul(
                        GB[:, nt - 2, :], AVT[:, nt * 128 : (nt + 1) * 128], WB[:],
                        start=True, stop=True,
                    )


                T_ = scratch.tile([128, NT, 4 * H], BF16, name="T", tag="T")
                FC = scratch.tile([128, NT, H], FP32, name="FC", tag="FC")
                IG = scratch.tile([128, NT, H], BF16, name="IG", tag="IG")
                TC = scratch.tile([128, NT, H], BF16, name="TC", tag="TC")

                # ACT queue order: tanhA, tanhB, TCA, TCB
                nc.scalar.activation(
                    T_[:, 0:2, 0 : 2 * H], GA[:, :, 0 : 2 * H], TANH, scale=0.5
                )
                nc.scalar.activation(
                    T_[:, 0:2, 3 * H : 4 * H], GA[:, :, 3 * H : 4 * H], TANH,
                    scale=0.5,
                )
                nc.scalar.activation(
                    T_[:, 0:2, 2 * H : 3 * H], GA[:, :, 2 * H : 3 * H], TANH,
                    scale=0.5,
                )
                nc.scalar.activation(T_[:, 2:NT, :], GB[:, :, :], TANH, scale=0.5)

                # half A tail (STT is DVE-only: Pool rejects TensorScalarPtr)
                nc.vector.scalar_tensor_tensor(
                    FC[:, 0:2, :], T_[:, 0:2, H : 2 * H], 1.0, C2[:, 0:2, :],
                    ADD, MUL,
                )
                nc.vector.scalar_tensor_tensor(
                    IG[:, 0:2, :], T_[:, 0:2, 0:H], 1.0, T_[:, 0:2, 3 * H : 4 * H],
                    ADD, MUL,
                )
                nc.vector.scalar_tensor_tensor(
                    C2[:, 0:2, :], FC[:, 0:2, :], 0.5, IG[:, 0:2, :], MUL, ADD,
                )
                # PE warmers pinned to this step via AVT/T_ reads: keep the
                # PE clock-gate warm through the elementwise tail
                nc.tensor.matmul(
                    JP[:, 0:N], JS[0:F, 0:128], AVT[0:F, :], start=True, stop=True,
                )
                for _ in range(4):
                    nc.tensor.matmul(
                        JP[:], JS[:, 0:128], T_[:, 0:2, :], start=True, stop=True,
                    )
                nc.scalar.activation(TC[:, 0:2, :], C2[:, 0:2, :], TANH, scale=0.5)
                # half B products fill DVE while ACT computes TC of half A
                nc.vector.scalar_tensor_tensor(
                    FC[:, 2:NT, :], T_[:, 2:NT, H : 2 * H], 1.0, C2[:, 2:NT, :],
                    ADD, MUL,
                )
                nc.vector.scalar_tensor_tensor(
                    IG[:, 2:NT, :], T_[:, 2:NT, 0:H], 1.0, T_[:, 2:NT, 3 * H : 4 * H],
                    ADD, MUL,
                )
                hdstA = HF[:, 0:2, :] if s == S - 1 else V[:, 0:2, 0:H]
                hdstB = HF[:, 2:NT, :] if s == S - 1 else V[:, 2:NT, 0:H]
                # hA as early as possible: it gates next step's mm1 mt0/mt1
                nc.vector.scalar_tensor_tensor(
                    hdstA, T_[:, 0:2, 2 * H : 3 * H], 1.0, TC[:, 0:2, :], ADD, MUL,
                )
                nc.vector.scalar_tensor_tensor(
                    C2[:, 2:NT, :], FC[:, 2:NT, :], 0.5, IG[:, 2:NT, :], MUL, ADD,
                )
                nc.scalar.activation(TC[:, 2:NT, :], C2[:, 2:NT, :], TANH, scale=0.5)
                nc.vector.scalar_tensor_tensor(
                    hdstB, T_[:, 2:NT, 2 * H : 3 * H], 1.0, TC[:, 2:NT, :], ADD, MUL,
                )
                for _ in range(2):  # bridge warmers gated on TC of half A
                    nc.tensor.matmul(
                        JP[:, 0:128], JS[:, 0:128], TC[:, 0:2, :],
                        start=True, stop=True,
                    )

            nc.sync.dma_start(hout_d[:], HF[:])

    nc.finalize()  # Bacc.finalize runs the multi-wait-splitting passes
    return nc


def _prep_core_inputs(b, x, adj, h0, c0, Wh, Wx, b_gates):
    bf16 = ml_dtypes.bfloat16
    # adjT[s, p, mt, n] = adj[b, s, n, mt*128+p]  (= A_s^T row m, col n)
    a = adj[b].transpose(0, 2, 1).reshape(S, NT, 128, N).transpose(0, 2, 1, 3)
    adjT = np.ascontiguousarray(a, dtype=bf16)
    # xT[p, s, mt, d] = x[b, s, mt*128+p, d]
    xb = x[b].reshape(S, NT, 128, D).transpose(2, 0, 1, 3)
    xT = np.ascontiguousarray(xb, dtype=bf16)
    # h0T/c0T[p, nt, j] = 2 * state[b, nt*128+p, j]  (states kept scaled x2)
    h0b = 2.0 * h0[b].reshape(NT, 128, H).transpose(1, 0, 2)
    c0b = 2.0 * c0[b].reshape(NT, 128, H).transpose(1, 0, 2)
    h0T = np.ascontiguousarray(h0b, dtype=bf16)
    c0T = np.ascontiguousarray(c0b, dtype=np.float32)
    # tanh-trick weight prep (all scalings are powers of 2 -> exact in bf16):
    #   h input arrives as 2h           -> Wh rows * 0.5
    #   gg block needs doubled preact   -> cols 3H:4H * 2
    wb = np.concatenate([Wh, Wx, b_gates[None, :]], axis=0).astype(np.float32)
    wb[0:H, :] *= 0.5
    wb[:, 3 * H : 4 * H] *= 2.0
    wb16 = wb.astype(bf16)
    return {"adjT": adjT, "xT": xT, "wb": wb16, "h0T": h0T, "c0T": c0T}


def _prep_all(inputs):
    x = np.asarray(inputs["x"], dtype=np.float32)
    adj = np.asarray(inputs["adj"], dtype=np.float32)
    h0 = np.asarray(inputs["initial_hidden_state"], dtype=np.float32)
    c0 = np.asarray(inputs["initial_cell_state"], dtype=np.float32)
    Wx_ = np.asarray(inputs["Wx"], dtype=np.float32)
    Wh_ = np.asarray(inputs["Wh"], dtype=np.float32)
    bg = np.asarray(inputs["b_gates"], dtype=np.float32)
    return [_prep_core_inputs(b, x, adj, h0, c0, Wh_, Wx_, bg) for b in range(B)]


def kernel(x, adj, initial_hidden_state, initial_cell_state, Wx, Wh, b_gates,
           W1, b1, W2, b2):
    if "nc" not in _CACHE:
        _CACHE["nc"] = _build_nc()
    nc = _CACHE["nc"]

    in_maps = _prep_all({
        "x": x, "adj": adj,
        "initial_hidden_state": initial_hidden_state,
        "initial_cell_state": initial_cell_state,
        "Wx": Wx, "Wh": Wh, "b_gates": b_gates,
    })
    res = run_bass_kernel_spmd(nc, in_maps, list(range(B)))

    h_final = np.zeros((B, N, H), dtype=np.float32)
    for i in range(B):
        hout = np.asarray(res.results[i]["hout"], dtype=np.float32)  # [128, NT, H]
        h_final[i] = 0.5 * hout.transpose(1, 0, 2).reshape(N, H)  # undo x2 scale

    read_out = h_final[:, 0, :]  # (B, H) -- TARGET_NODE = 0
    pre = read_out @ np.asarray(W1, dtype=np.float32) + np.asarray(b1, dtype=np.float32)
    out = np.maximum(pre, 0.0) @ np.asarray(W2, dtype=np.float32) + np.asarray(
        b2, dtype=np.float32
    )
    return out.astype(np.float32)


# revision 14
# speedup vs baseline: 1.0015x; 1.0015x over previous
import sys

sys.path.insert(0, "/opt/trn_rl_repo")

import numpy as np
import ml_dtypes

from concourse import bass, bacc, tile, mybir
from concourse.bass_utils import run_bass_kernel_spmd

B, S, N, D = 4, 96, 512, 8
H = 64
OUT = 24
NT = N // 128  # 4 node tiles of 128 partitions
F = H + D     # 72 features in v = [h | x]
FB = F + 1    # +1 ones row for bias

BF16 = mybir.dt.bfloat16
FP32 = mybir.dt.float32

_CACHE = {}

# All state is kept scaled by 2 (V holds 2h, C2 holds 2c) so that the whole
# LSTM tail runs off a single tanh per gate block:
#   sigmoid(x) = (tanh(x/2) + 1) / 2
# One ACT tanh(G*0.5) yields t_i,t_f,t_o,tanh(gg) at once (gg weight cols are
# pre-doubled on the host), and the (t+1)/2 factors fold into the
# scalar_tensor_tensor ops:
#   2c' = 0.5*[(t_f+1)*2c] + (t_i+1)*t_g
#   2h' = (t_o+1)*tanh(2c' * 0.5)


def _build_nc():
    nc = bacc.Bacc(None)
    adjT_d = nc.dram_tensor("adjT", [S, 128, NT, N], BF16, kind="ExternalInput")
    xT_d = nc.dram_tensor("xT", [128, S, NT, D], BF16, kind="ExternalInput")
    wb_d = nc.dram_tensor("wb", [FB, 4 * H], BF16, kind="ExternalInput")
    h0_d = nc.dram_tensor("h0T", [128, NT, H], BF16, kind="ExternalInput")
    c0_d = nc.dram_tensor("c0T", [128, NT, H], FP32, kind="ExternalInput")
    hout_d = nc.dram_tensor("hout", [128, NT, H], FP32, kind="ExternalOutput")

    TANH = mybir.ActivationFunctionType.Tanh
    COPY = mybir.ActivationFunctionType.Copy
    ADD = mybir.AluOpType.add
    MUL = mybir.AluOpType.mult

    with tile.TileContext(nc) as tc:
        with (
            tc.tile_pool(name="persist", bufs=1) as persist,
            tc.tile_pool(name="adj", bufs=3) as adjp,
            tc.tile_pool(name="scratch", bufs=2) as scratch,
            tc.tile_pool(name="ps_av", bufs=1, space="PSUM") as ps_av,
            tc.tile_pool(name="ps_g", bufs=2, space="PSUM") as ps_g,
            tc.tile_pool(name="ps_junk", bufs=1, space="PSUM") as ps_junk,
            tc.tile_pool(name="ps_c", bufs=1, space="PSUM") as ps_c,
        ):
            X = persist.tile([128, S, NT, D], BF16)   # all timesteps of x
            V = persist.tile([128, NT, F], BF16)      # [2h | x] per node tile
            C2 = ps_c.tile([128, NT, H], FP32)        # 2*cell state (PSUM:
            # ACT reads PSUM faster than SBUF-fp32, and TC->h is on the chain)
            WB = persist.tile([FB, 4 * H], BF16)      # [Wh'; Wx'; b']
            AVT = persist.tile([FB, N], BF16)         # Av^T + ones row
            HF = persist.tile([128, NT, H], FP32)     # final 2h, fp32
            H0 = persist.tile([128, NT, H], BF16)
            JS = persist.tile([128, 256], BF16)       # warmer matmul source
            JP = ps_junk.tile([128, 512], FP32)       # warmer matmul sink

            nc.gpsimd.dma_start(X[:], xT_d[:])
            nc.gpsimd.dma_start(WB[:], wb_d[:])
            nc.gpsimd.dma_start(H0[:], h0_d[:])
            C0S = persist.tile([128, NT, H], FP32)
            nc.gpsimd.dma_start(C0S[:], c0_d[:])
            nc.vector.tensor_copy(C2[:], C0S[:])  # stage c0 via SBUF into PSUM
            # all V producers stay on DVE so matmul LDW needs a single wait
            nc.vector.tensor_copy(V[:, :, 0:H], H0[:])
            nc.vector.memset(JS[:], 0.0)
            # ones row (72) for bias; partition offset must be mult of 32, so
            # memset 64:73 once — rows 64:72 are rewritten with data each step.
            nc.vector.memset(AVT[64:FB, :], 1.0)

            for s in range(S):
                AT = adjp.tile([128, NT, N], BF16, name="AT", tag="AT")
                nc.sync.dma_start(AT[:], adjT_d[s])

                # x_s into V x slots (SBUF -> SBUF), on Pool to keep DVE free
                nc.gpsimd.tensor_copy(V[:, :, H : H + D], X[:, s, :, :])

                AvT0 = ps_av.tile([FB, 256], FP32, name="AvT0", tag="AvT0")
                AvT1 = ps_av.tile([FB, 256], FP32, name="AvT1", tag="AvT1")
                GA = ps_g.tile([128, 2, 4 * H], FP32, name="GA", tag="GA")
                GB = ps_g.tile([128, 2, 4 * H], FP32, name="GB", tag="GB")

                # mm1 col-half 0; mt0/1 start as soon as h tiles 0:2 land,
                # mt2/3 wait on h tiles 2:4 (warmers below bridge the gap)
                for mt in range(2):
                    nc.tensor.matmul(
                        AvT0[0:F, :], V[:, mt, :], AT[:, mt, 0:256],
                        start=(mt == 0), stop=False,
                    )
                for mt in range(2, NT):
                    nc.tensor.matmul(
                        AvT0[0:F, :], V[:, mt, :], AT[:, mt, 0:256],
                        start=False, stop=(mt == NT - 1),
                    )
                for _ in range(2):
                    nc.tensor.matmul(
                        JP[:, 0:16], JS[:, 0:128], V[:, 3, 0:16],
                        start=True, stop=True,
                    )
                nc.vector.tensor_copy(AVT[0:F, 0:256], AvT0[0:F, :])
                for mt in range(NT):
                    nc.tensor.matmul(
                        AvT1[0:F, :], V[:, mt, :], AT[:, mt, 256:512],
                        start=(mt == 0), stop=(mt == NT - 1),
                    )
                nc.scalar.activation(AVT[0:F, 256:512], AvT1[0:F, :], COPY)

                for nt in range(2):
                    nc.tensor.matmul(
                        GA[:, nt, :], AVT[:, nt * 128 : (nt + 1) * 128], WB[:],
                        start=True, stop=True,
                    )
                for nt in range(2, NT):
                    nc.tensor.matmul(
                        GB[:, nt - 2, :], AVT[:, nt * 128 : (nt + 1) * 128], WB[:],
                        start=True, stop=True,
                    )


                T_ = scratch.tile([128, NT, 4 * H], BF16, name="T", tag="T")
                FC = scratch.tile([128, NT, H], FP32, name="FC", tag="FC")
                IG = scratch.tile([128, NT, H], BF16, name="IG", tag="IG")
                TC = scratch.tile([128, NT, H], BF16, name="TC", tag="TC")

                # ACT queue order: tanhA, tanhB, TCA, TCB
                nc.scalar.activation(T_[:, 0:2, :], GA[:, :, :], TANH, scale=0.5)
                nc.scalar.activation(T_[:, 2:NT, :], GB[:, :, :], TANH, scale=0.5)

                # half A tail (STT is DVE-only: Pool rejects TensorScalarPtr)
                nc.vector.scalar_tensor_tensor(
                    FC[:, 0:2, :], T_[:, 0:2, H : 2 * H], 1.0, C2[:, 0:2, :],
                    ADD, MUL,
                )
                nc.vector.scalar_tensor_tensor(
                    IG[:, 0:2, :], T_[:, 0:2, 0:H], 1.0, T_[:, 0:2, 3 * H : 4 * H],
                    ADD, MUL,
                )
                nc.vector.scalar_tensor_tensor(
                    C2[:, 0:2, :], FC[:, 0:2, :], 0.5, IG[:, 0:2, :], MUL, ADD,
                )
                # PE warmers pinned to this step via AVT/T_ reads: keep the
                # PE clock-gate warm through the elementwise tail
                nc.tensor.matmul(
                    JP[:, 0:N], JS[0:F, 0:128], AVT[0:F, :], start=True, stop=True,
                )
                for _ in range(4):
                    nc.tensor.matmul(
                        JP[:], JS[:, 0:128], T_[:, 0:2, :], start=True, stop=True,
                    )
                nc.scalar.activation(TC[:, 0:2, :], C2[:, 0:2, :], TANH, scale=0.5)
                # half B products fill DVE while ACT computes TC of half A
                nc.vector.scalar_tensor_tensor(
                    FC[:, 2:NT, :], T_[:, 2:NT, H : 2 * H], 1.0, C2[:, 2:NT, :],
                    ADD, MUL,
                )
                nc.vector.scalar_tensor_tensor(
                    IG[:, 2:NT, :], T_[:, 2:NT, 0:H], 1.0, T_[:, 2:NT, 3 * H : 4 * H],
                    ADD, MUL,
                )
                hdstA = HF[:, 0:2, :] if s == S - 1 else V[:, 0:2, 0:H]
                hdstB = HF[:, 2:NT, :] if s == S - 1 else V[:, 2:NT, 0:H]
                # hA as early as possible: it gates next step's mm1 mt0/mt1
                nc.vector.scalar_tensor_tensor(
                    hdstA, T_[:, 0:2, 2 * H : 3 * H], 1.0, TC[:, 0:2, :], ADD, MUL,
                )
                nc.vector.scalar_tensor_tensor(
                    C2[:, 2:NT, :], FC[:, 2:NT, :], 0.5, IG[:, 2:NT, :], MUL, ADD,
                )
                nc.scalar.activation(TC[:, 2:NT, :], C2[:, 2:NT, :], TANH, scale=0.5)
                nc.vector.scalar_tensor_tensor(
                    hdstB, T_[:, 2:NT, 2 * H : 3 * H], 1.0, TC[:, 2:NT, :], ADD, MUL,
                )
                for _ in range(2):  # bridge warmers gated on TC of half A
                    nc.tensor.matmul(
                        JP[:, 0:128], JS[:, 0:128], TC[:, 0:2, :],
                        start=True, stop=True,
                    )

            nc.sync.dma_start(hout_d[:], HF[:])

    nc.finalize()  # Bacc.finalize runs the multi-wait-splitting passes
    return nc


def _prep_core_inputs(b, x, adj, h0, c0, Wh, Wx, b_gates):
    bf16 = ml_dtypes.bfloat16
    # adjT[s, p, mt, n] = adj[b, s, n, mt*128+p]  (= A_s^T row m, col n)
    a = adj[b].transpose(0, 2, 1).reshape(S, NT, 128, N).transpose(0, 2, 1, 3)
    adjT = np.ascontiguousarray(a, dtype=bf16)
    # xT[p, s, mt, d] = x[b, s, mt*128+p, d]
    xb = x[b].reshape(S, NT, 128, D).transpose(2, 0, 1, 3)
    xT = np.ascontiguousarray(xb, dtype=bf16)
    # h0T/c0T[p, nt, j] = 2 * state[b, nt*128+p, j]  (states kept scaled x2)
    h0b = 2.0 * h0[b].reshape(NT, 128, H).transpose(1, 0, 2)
    c0b = 2.0 * c0[b].reshape(NT, 128, H).transpose(1, 0, 2)
    h0T = np.ascontiguousarray(h0b, dtype=bf16)
    c0T = np.ascontiguousarray(c0b, dtype=np.float32)
    # tanh-trick weight prep (all scalings are powers of 2 -> exact in bf16):
    #   h input arrives as 2h           -> Wh rows * 0.5
    #   gg block needs doubled preact   -> cols 3H:4H * 2
    wb = np.concatenate([Wh, Wx, b_gates[None, :]], axis=0).astype(np.float32)
    wb[0:H, :] *= 0.5
    wb[:, 3 * H : 4 * H] *= 2.0
    wb16 = wb.astype(bf16)
    return {"adjT": adjT, "xT": xT, "wb": wb16, "h0T": h0T, "c0T": c0T}


def _prep_all(inputs):
    x = np.asarray(inputs["x"], dtype=np.float32)
    adj = np.asarray(inputs["adj"], dtype=np.float32)
    h0 = np.asarray(inputs["initial_hidden_state"], dtype=np.float32)
    c0 = np.asarray(inputs["initial_cell_state"], dtype=np.float32)
    Wx_ = np.asarray(inputs["Wx"], dtype=np.float32)
    Wh_ = np.asarray(inputs["Wh"], dtype=np.float32)
    bg = np.asarray(inputs["b_gates"], dtype=np.float32)
    return [_prep_core_inputs(b, x, adj, h0, c0, Wh_, Wx_, bg) for b in range(B)]


def kernel(x, adj, initial_hidden_state, initial_cell_state, Wx, Wh, b_gates,
           W1, b1, W2, b2):
    if "nc" not in _CACHE:
        _CACHE["nc"] = _build_nc()
    nc = _CACHE["nc"]

    in_maps = _prep_all({
        "x": x, "adj": adj,
        "initial_hidden_state": initial_hidden_state,
        "initial_cell_state": initial_cell_state,
        "Wx": Wx, "Wh": Wh, "b_gates": b_gates,
    })
    res = run_bass_kernel_spmd(nc, in_maps, list(range(B)))

    h_final = np.zeros((B, N, H), dtype=np.float32)
    for i in range(B):
        hout = np.asarray(res.results[i]["hout"], dtype=np.float32)  # [128, NT, H]
        h_final[i] = 0.5 * hout.transpose(1, 0, 2).reshape(N, H)  # undo x2 scale

    read_out = h_final[:, 0, :]  # (B, H) -- TARGET_NODE = 0
    pre = read_out @ np.asarray(W1, dtype=np.float32) + np.asarray(b1, dtype=np.float32)
    out = np.maximum(pre, 0.0) @ np.asarray(W2, dtype=np.float32) + np.asarray(
        b2, dtype=np.float32
    )
    return out.astype(np.float32)


# revision 15
# speedup vs baseline: 1.3258x; 1.3238x over previous
import sys

sys.path.insert(0, "/opt/trn_rl_repo")

import numpy as np
import ml_dtypes

from concourse import bass, bacc, tile, mybir
from concourse.bass_utils import run_bass_kernel_spmd

B, S, N, D = 4, 96, 512, 8
H = 64
OUT = 24
NT = N // 128  # 4 node tiles of 128 partitions
F = H + D     # 72 features in v = [h | x]
FB = F + 1    # +1 ones row for bias

BF16 = mybir.dt.bfloat16
FP32 = mybir.dt.float32

_CACHE = {}

# All state is kept scaled by 2 (V holds 2h, C2 holds 2c) so that the whole
# LSTM tail runs off a single tanh per gate block:
#   sigmoid(x) = (tanh(x/2) + 1) / 2
# One ACT tanh(G*0.5) yields t_i,t_f,t_o,tanh(gg) at once (gg weight cols are
# pre-doubled on the host), and the (t+1)/2 factors fold into the
# scalar_tensor_tensor ops:
#   2c' = 0.5*[(t_f+1)*2c] + (t_i+1)*t_g
#   2h' = (t_o+1)*tanh(2c' * 0.5)


def _build_nc():
    nc = bacc.Bacc(None)
    adjT_d = nc.dram_tensor("adjT", [S, 128, NT, N], BF16, kind="ExternalInput")
    xT_d = nc.dram_tensor("xT", [128, S, NT, D], BF16, kind="ExternalInput")
    wb_d = nc.dram_tensor("wb", [FB, 4 * H], BF16, kind="ExternalInput")
    h0_d = nc.dram_tensor("h0T", [128, NT, H], BF16, kind="ExternalInput")
    c0_d = nc.dram_tensor("c0T", [128, NT, H], FP32, kind="ExternalInput")
    hout_d = nc.dram_tensor("hout", [128, NT, H], FP32, kind="ExternalOutput")

    TANH = mybir.ActivationFunctionType.Tanh
    COPY = mybir.ActivationFunctionType.Copy
    ADD = mybir.AluOpType.add
    MUL = mybir.AluOpType.mult

    with tile.TileContext(nc) as tc:
        with (
            tc.tile_pool(name="persist", bufs=1) as persist,
            tc.tile_pool(name="adj", bufs=3) as adjp,
            tc.tile_pool(name="scratch", bufs=2) as scratch,
            tc.tile_pool(name="ps_av", bufs=1, space="PSUM") as ps_av,
            tc.tile_pool(name="ps_g", bufs=2, space="PSUM") as ps_g,
            tc.tile_pool(name="ps_junk", bufs=1, space="PSUM") as ps_junk,
        ):
            X = persist.tile([128, S, NT, D], BF16)   # all timesteps of x
            V = persist.tile([128, NT, F], BF16)      # [2h | x] per node tile
            C2 = persist.tile([128, NT, H], FP32)     # 2*cell state
            WB = persist.tile([FB, 4 * H], BF16)      # [Wh'; Wx'; b']
            AVT = persist.tile([FB, N], BF16)         # Av^T + ones row
            HF = persist.tile([128, NT, H], FP32)     # final 2h, fp32
            H0 = persist.tile([128, NT, H], BF16)
            JS = persist.tile([128, 256], BF16)       # warmer matmul source
            JP = ps_junk.tile([128, 512], FP32)       # warmer matmul sink

            nc.gpsimd.dma_start(X[:], xT_d[:])
            nc.gpsimd.dma_start(WB[:], wb_d[:])
            nc.gpsimd.dma_start(H0[:], h0_d[:])
            nc.gpsimd.dma_start(C2[:], c0_d[:])
            # all V producers stay on DVE so matmul LDW needs a single wait
            nc.vector.tensor_copy(V[:, :, 0:H], H0[:])
            nc.vector.memset(JS[:], 0.0)
            # ones row (72) for bias; partition offset must be mult of 32, so
            # memset 64:73 once — rows 64:72 are rewritten with data each step.
            nc.vector.memset(AVT[64:FB, :], 1.0)

            for s in range(S):
                AT = adjp.tile([128, NT, N], BF16, name="AT", tag="AT")
                nc.sync.dma_start(AT[:], adjT_d[s])

                # x_s into V x slots (SBUF -> SBUF), on Pool to keep DVE free
                nc.gpsimd.tensor_copy(V[:, :, H : H + D], X[:, s, :, :])

                AvT0 = ps_av.tile([FB, 256], FP32, name="AvT0", tag="AvT0")
                AvT1 = ps_av.tile([FB, 256], FP32, name="AvT1", tag="AvT1")
                GA = ps_g.tile([128, 2, 4 * H], FP32, name="GA", tag="GA")
                GB = ps_g.tile([128, 2, 4 * H], FP32, name="GB", tag="GB")

                # mm1 col-half 0; mt0/1 start as soon as h tiles 0:2 land,
                # mt2/3 wait on h tiles 2:4 (warmers below bridge the gap)
                for mt in range(2):
                    nc.tensor.matmul(
                        AvT0[0:F, :], V[:, mt, :], AT[:, mt, 0:256],
                        start=(mt == 0), stop=False,
                    )
                for mt in range(2, NT):
                    nc.tensor.matmul(
                        AvT0[0:F, :], V[:, mt, :], AT[:, mt, 0:256],
                        start=False, stop=(mt == NT - 1),
                    )
                for _ in range(2):
                    nc.tensor.matmul(
                        JP[:, 0:16], JS[:, 0:128], V[:, 3, 0:16],
                        start=True, stop=True,
                    )
                nc.vector.tensor_copy(AVT[0:F, 0:256], AvT0[0:F, :])
                for mt in range(NT):
                    nc.tensor.matmul(
                        AvT1[0:F, :], V[:, mt, :], AT[:, mt, 256:512],
                        start=(mt == 0), stop=(mt == NT - 1),
                    )
                nc.scalar.activation(AVT[0:F, 256:512], AvT1[0:F, :], COPY)

                for nt in range(2):
                    nc.tensor.matmul(
                        GA[:, nt, :], AVT[:, nt * 128 : (nt + 1) * 128], WB[:],
                        start=True, stop=True,
                    )
                for nt in range(2, NT):
                    nc.tensor.matmul(
                        GB[:, nt - 2, :], AVT[:, nt * 128 : (nt + 1) * 128], WB[:],
                        start=True, stop=True,
                    )


                T_ = scratch.tile([128, NT, 4 * H], BF16, name="T", tag="T")
                FC = scratch.tile([128, NT, H], FP32, name="FC", tag="FC")
                IG = scratch.tile([128, NT, H], BF16, name="IG", tag="IG")
                TC = scratch.tile([128, NT, H], BF16, name="TC", tag="TC")

                # ACT queue order: tanhA, tanhB, TCA, TCB
                nc.scalar.activation(T_[:, 0:2, :], GA[:, :, :], TANH, scale=0.5)
                nc.scalar.activation(T_[:, 2:NT, :], GB[:, :, :], TANH, scale=0.5)

                # half A tail (STT is DVE-only: Pool rejects TensorScalarPtr)
                nc.vector.scalar_tensor_tensor(
                    FC[:, 0:2, :], T_[:, 0:2, H : 2 * H], 1.0, C2[:, 0:2, :],
                    ADD, MUL,
                )
                nc.vector.scalar_tensor_tensor(
                    IG[:, 0:2, :], T_[:, 0:2, 0:H], 1.0, T_[:, 0:2, 3 * H : 4 * H],
                    ADD, MUL,
                )
                nc.vector.scalar_tensor_tensor(
                    C2[:, 0:2, :], FC[:, 0:2, :], 0.5, IG[:, 0:2, :], MUL, ADD,
                )
                # PE warmers pinned to this step via AVT/T_ reads: keep the
                # PE clock-gate warm through the elementwise tail
                nc.tensor.matmul(
                    JP[:, 0:N], JS[0:F, 0:128], AVT[0:F, :], start=True, stop=True,
                )
                for _ in range(4):
                    nc.tensor.matmul(
                        JP[:], JS[:, 0:128], T_[:, 0:2, :], start=True, stop=True,
                    )
                nc.scalar.activation(TC[:, 0:2, :], C2[:, 0:2, :], TANH, scale=0.5)
                # half B products fill DVE while ACT computes TC of half A
                nc.vector.scalar_tensor_tensor(
                    FC[:, 2:NT, :], T_[:, 2:NT, H : 2 * H], 1.0, C2[:, 2:NT, :],
                    ADD, MUL,
                )
                nc.vector.scalar_tensor_tensor(
                    IG[:, 2:NT, :], T_[:, 2:NT, 0:H], 1.0, T_[:, 2:NT, 3 * H : 4 * H],
                    ADD, MUL,
                )
                hdstA = HF[:, 0:2, :] if s == S - 1 else V[:, 0:2, 0:H]
                hdstB = HF[:, 2:NT, :] if s == S - 1 else V[:, 2:NT, 0:H]
                # hA as early as possible: it gates next step's mm1 mt0/mt1
                nc.vector.scalar_tensor_tensor(
                    hdstA, T_[:, 0:2, 2 * H : 3 * H], 1.0, TC[:, 0:2, :], ADD, MUL,
                )
                nc.vector.scalar_tensor_tensor(
                    C2[:, 2:NT, :], FC[:, 2:NT, :], 0.5, IG[:, 2:NT, :], MUL, ADD,
                )
                nc.scalar.activation(TC[:, 2:NT, :], C2[:, 2:NT, :], TANH, scale=0.5)
                nc.vector.scalar_tensor_tensor(
                    hdstB, T_[:, 2:NT, 2 * H : 3 * H], 1.0, TC[:, 2:NT, :], ADD, MUL,
                )
                for _ in range(2):  # bridge warmers gated on TC of half A
                    nc.tensor.matmul(
                        JP[:, 0:128], JS[:, 0:128], TC[:, 0:2, :],
                        start=True, stop=True,
                    )

            nc.sync.dma_start(hout_d[:], HF[:])

    nc.finalize()  # Bacc.finalize runs the multi-wait-splitting passes
    return nc


def _prep_core_inputs(b, x, adj, h0, c0, Wh, Wx, b_gates):
    bf16 = ml_dtypes.bfloat16
    # adjT[s, p, mt, n] = adj[b, s, n, mt*128+p]  (= A_s^T row m, col n)
    a = adj[b].transpose(0, 2, 1).reshape(S, NT, 128, N).transpose(0, 2, 1, 3)
    adjT = np.ascontiguousarray(a, dtype=bf16)
    # xT[p, s, mt, d] = x[b, s, mt*128+p, d]
    xb = x[b].reshape(S, NT, 128, D).transpose(2, 0, 1, 3)
    xT = np.ascontiguousarray(xb, dtype=bf16)
    # h0T/c0T[p, nt, j] = 2 * state[b, nt*128+p, j]  (states kept scaled x2)
    h0b = 2.0 * h0[b].reshape(NT, 128, H).transpose(1, 0, 2)
    c0b = 2.0 * c0[b].reshape(NT, 128, H).transpose(1, 0, 2)
    h0T = np.ascontiguousarray(h0b, dtype=bf16)
    c0T = np.ascontiguousarray(c0b, dtype=np.float32)
    # tanh-trick weight prep (all scalings are powers of 2 -> exact in bf16):
    #   h input arrives as 2h           -> Wh rows * 0.5
    #   gg block needs doubled preact   -> cols 3H:4H * 2
    wb = np.concatenate([Wh, Wx, b_gates[None, :]], axis=0).astype(np.float32)
    wb[0:H, :] *= 0.5
    wb[:, 3 * H : 4 * H] *= 2.0
    wb16 = wb.astype(bf16)
    return {"adjT": adjT, "xT": xT, "wb": wb16, "h0T": h0T, "c0T": c0T}


def _prep_all(inputs):
    x = np.asarray(inputs["x"], dtype=np.float32)
    adj = np.asarray(inputs["adj"], dtype=np.float32)
    h0 = np.asarray(inputs["initial_hidden_state"], dtype=np.float32)
    c0 = np.asarray(inputs["initial_cell_state"], dtype=np.float32)
    Wx_ = np.asarray(inputs["Wx"], dtype=np.float32)
    Wh_ = np.asarray(inputs["Wh"], dtype=np.float32)
    bg = np.asarray(inputs["b_gates"], dtype=np.float32)
    return [_prep_core_inputs(b, x, adj, h0, c0, Wh_, Wx_, bg) for b in range(B)]


def kernel(x, adj, initial_hidden_state, initial_cell_state, Wx, Wh, b_gates,
           W1, b1, W2, b2):
    if "nc" not in _CACHE:
        _CACHE["nc"] = _build_nc()
    nc = _CACHE["nc"]

    in_maps = _prep_all({
        "x": x, "adj": adj,
        "initial_hidden_state": initial_hidden_state,
        "initial_cell_state": initial_cell_state,
        "Wx": Wx, "Wh": Wh, "b_gates": b_gates,
    })
    res = run_bass_kernel_spmd(nc, in_maps, list(range(B)))

    h_final = np.zeros((B, N, H), dtype=np.float32)
    for i in range(B):
        hout = np.asarray(res.results[i]["hout"], dtype=np.float32)  # [128, NT, H]
        h_final[i] = 0.5 * hout.transpose(1, 0, 2).reshape(N, H)  # undo x2 scale

    read_out = h_final[:, 0, :]  # (B, H) -- TARGET_NODE = 0
    pre = read_out @ np.asarray(W1, dtype=np.float32) + np.asarray(b1, dtype=np.float32)
    out = np.maximum(pre, 0.0) @ np.asarray(W2, dtype=np.float32) + np.asarray(
        b2, dtype=np.float32
    )
    return out.astype(np.float32)
